# revision 47
# baseline (speedup 1.0000x reference)
"""Trainium2 Bass kernel: causal depthwise Conv1d (K=4) + SiLU.

Reference computation (B=4, S=4096, D=2048):
    y[b, s, d] = silu( sum_k w[d, 0, k] * x[b, s-3+k, d] )   (zero-padded left)

Strategy (variant "pe"):
  * Host: transpose x to channel-major (D, B, S), left-pad each row with
    4 zeros (row length 4100), cast to bf16, shard D across the 8
    NeuronCores (256 channels each).  Depthwise conv is channel-independent
    -> no inter-core communication.
  * Core: the conv runs entirely on the TensorEngine: a matmul with a
    DIAGONAL stationary matrix diag(w[:,k]) applied to a shifted slice of
    the x tile computes w_k[d] * x[d, s+k]; the 4 taps accumulate in PSUM
    (fp32).  ACT applies native Silu reading PSUM directly, writes bf16.
  * Host: gather, transpose back, cast to f32.

"""

import os
import sys

sys.path.insert(0, "/opt/trn_rl_repo")

import numpy as np
import ml_dtypes

N_CORES = 8
B, S, D = 4, 4096, 2048
K = 4
PAD = 4
ROW = S + PAD  # 4100
D_LOCAL = D // N_CORES  # 256
G = D_LOCAL // 128  # 2 partition groups per core

MM_N = int(os.environ.get("KERNEL_MM_N", "512"))
N_DVE_TILES = int(os.environ.get("KERNEL_N_DVE", "3"))

_CACHE = {}


def _build_pe():
    """Hybrid: PE (diag-stationary matmul, PSUM accumulate) on 5 tiles,
    DVE full chain on 3 tiles, ACT only does Silu."""
    import concourse.tile as tile
    from concourse import bacc, mybir

    nc = bacc.Bacc("TRN2", debug=False, enable_asserts=False, num_devices=N_CORES)
    bf16 = mybir.dt.bfloat16
    f32 = mybir.dt.float32

    x_ap = nc.dram_tensor("x", [G, 128, B, ROW], bf16, kind="ExternalInput").ap()
    wd_ap = nc.dram_tensor("wd", [128, G * K * 128], bf16, kind="ExternalInput").ap()
    w_ap = nc.dram_tensor("w", [128, G * K], f32, kind="ExternalInput").ap()
    out_ap = nc.dram_tensor("out", [G, 128, B, S], bf16, kind="ExternalOutput").ap()

    dve_tiles = {
        0: set(), 1: {1}, 2: {1, 4}, 3: {1, 4, 6}, 4: {1, 3, 5, 7},
    }[N_DVE_TILES]

    with tile.TileContext(nc) as tc:
        with (
            tc.tile_pool(name="wp", bufs=1) as wp,
            tc.tile_pool(name="xp", bufs=6) as xp,
            tc.tile_pool(name="tp", bufs=2) as tp,
            tc.tile_pool(name="cp", bufs=2) as cp,
            tc.tile_pool(name="ps", bufs=2, space="PSUM") as ps,
            tc.tile_pool(name="yp", bufs=4) as yp,
        ):
            # small weight DMAs first on the sync queue so LDWEIGHTS and the
            # DVE tap-muls can start as soon as the first x tile lands
            wd = wp.tile([128, G * K * 128], bf16, tag="wd")
            nc.gpsimd.dma_start(out=wd[:], in_=wd_ap[:])
            wt = wp.tile([128, G * K], f32, tag="wt")
            nc.gpsimd.dma_start(out=wt[:], in_=w_ap[:])

            def wdiag(g, k):
                c0 = (g * K + k) * 128
                return wd[:, c0 : c0 + 128]

            def emit_dve(g, b, xt, lo=0, hi=S, prio_bump=80):
                # y[s] = sum_k w_k * xt[s + 1 + k] on the vector engine;
                # misaligned bf16 tensor_scalar measured at full 4x on HW
                W = hi - lo

                def wcol(k):
                    return wt[:, g * K + k : g * K + k + 1]

                ts = []
                for k in range(K):
                    t = tp.tile([128, W], bf16, tag=f"t{k % 2}")
                    nc.vector.tensor_scalar_mul(
                        t[:], xt[:, lo + 1 + k : lo + 1 + k + W], wcol(k)
                    )
                    ts.append(t)
                p0 = cp.tile([128, W], bf16, tag="p0")
                nc.vector.tensor_add(p0[:], ts[0][:], ts[1][:])
                p1 = cp.tile([128, W], bf16, tag="p1")
                nc.vector.tensor_add(p1[:], ts[2][:], ts[3][:])
                c = cp.tile([128, W], bf16, tag="c")
                nc.vector.tensor_add(c[:], p0[:], p1[:])
                y = yp.tile([128, W], bf16, tag="y")
                # small silu chunks, deprioritized so PE's psum-draining
                # silus come first in ACT's static order (head-of-line)
                for c0 in range(0, W, 1024):
                    cw = min(1024, W - c0)
                    si = nc.scalar.activation(
                        out=y[:, c0 : c0 + cw],
                        in_=c[:, c0 : c0 + cw],
                        func=mybir.ActivationFunctionType.Silu,
                    )
                    si.ins.bass_priority = (si.ins.bass_priority or 0) + prio_bump
                for c0 in range(0, W, 2048):
                    cw = min(2048, W - c0)
                    nc.gpsimd.dma_start(
                        out=out_ap[g, :, b, lo + c0 : lo + c0 + cw],
                        in_=y[:, c0 : c0 + cw],
                    )

            def emit_pe(g, b, xt, lo=0, hi=S, last=False):
                y = yp.tile([128, hi - lo], bf16, tag="y")
                for c0 in range(lo, hi, 2048):
                    cw = min(2048, hi - c0)
                    acc = ps.tile([128, cw], f32, tag="acc")
                    for k in range(K):
                        for n0 in range(0, cw, MM_N):
                            xlo = c0 + n0 + 1 + k
                            nc.tensor.matmul(
                                acc[:, n0 : n0 + MM_N],
                                wdiag(g, k),
                                xt[:, xlo : xlo + MM_N],
                                start=(k == 0),
                                stop=(k == K - 1),
                            )
                    final_chunk = last and c0 + 2048 >= hi
                    if final_chunk:
                        # drain the tail at 1024 granularity on two queues
                        for i, s0 in enumerate(range(c0, c0 + cw, 1024)):
                            nc.scalar.activation(
                                out=y[:, s0 - lo : s0 - lo + 1024],
                                in_=acc[:, s0 - c0 : s0 - c0 + 1024],
                                func=mybir.ActivationFunctionType.Silu,
                            )
                            oeng = nc.gpsimd if i % 2 == 0 else nc.scalar
                            oeng.dma_start(
                                out=out_ap[g, :, b, s0 : s0 + 1024],
                                in_=y[:, s0 - lo : s0 - lo + 1024],
                            )
                    else:
                        nc.scalar.activation(
                            out=y[:, c0 - lo : c0 - lo + cw],
                            in_=acc[:],
                            func=mybir.ActivationFunctionType.Silu,
                        )
                        nc.gpsimd.dma_start(
                            out=out_ap[g, :, b, c0 : c0 + cw],
                            in_=y[:, c0 - lo : c0 - lo + cw],
                        )

            for g in range(G):
                for b in range(B):
                    tile_idx = g * B + b
                    xt = xp.tile([128, ROW], bf16, tag="xt")
                    if tile_idx == 0:
                        # split the first load: PE starts on the first half
                        h = 2052
                        nc.sync.dma_start(out=xt[:, 0:h], in_=x_ap[g, :, b, 0:h])
                        nc.sync.dma_start(
                            out=xt[:, h:ROW], in_=x_ap[g, :, b, h:ROW]
                        )
                    else:
                        nc.sync.dma_start(out=xt[:], in_=x_ap[g, :, b, :])

                    if tile_idx in dve_tiles:
                        # don't delay the final DVE tile's silus (tail path)
                        emit_dve(g, b, xt, prio_bump=80 if tile_idx < 6 else 0)
                    else:
                        emit_pe(g, b, xt, last=(tile_idx == G * B - 1))

    nc.compile()
    return nc


def _get_nc():
    if "nc" not in _CACHE:
        _CACHE["nc"] = _build_pe()
    return _CACHE["nc"]


def _make_in_maps(x, w):
    x = np.asarray(x, dtype=np.float32)
    w = np.asarray(w, dtype=np.float32)

    # (B, S, D) -> (D, B, S), bf16, left-pad rows with PAD zeros.
    x_t = np.ascontiguousarray(x.transpose(2, 0, 1)).astype(ml_dtypes.bfloat16)
    x_pad = np.zeros((D, B, ROW), dtype=ml_dtypes.bfloat16)
    x_pad[:, :, PAD:] = x_t
    w_flat = np.ascontiguousarray(w[:, 0, :])  # (D, K) f32

    in_maps = []
    for i in range(N_CORES):
        lo, hi = i * D_LOCAL, (i + 1) * D_LOCAL
        m = {"x": np.ascontiguousarray(x_pad[lo:hi].reshape(G, 128, B, ROW))}
        m["w"] = np.ascontiguousarray(
            w_flat[lo:hi].reshape(G, 128, K).transpose(1, 0, 2).reshape(128, G * K)
        )
        # diag stationaries, laid out [128, G*K*128] partition-first
        wd = np.zeros((G, K, 128, 128), dtype=ml_dtypes.bfloat16)
        wl = w_flat[lo:hi].reshape(G, 128, K).astype(ml_dtypes.bfloat16)
        idx = np.arange(128)
        for g in range(G):
            for k in range(K):
                wd[g, k, idx, idx] = wl[g, :, k]
        # (G,K,p,m) -> (p, G,K,m) -> [128, G*K*128]
        m["wd"] = np.ascontiguousarray(
            wd.transpose(2, 0, 1, 3).reshape(128, G * K * 128)
        )
        in_maps.append(m)
    return in_maps


def _assemble(results):
    parts = []
    for r in results:
        y = np.asarray(r["out"]).reshape(D_LOCAL, B, S)
        parts.append(y)
    y_full = np.concatenate(parts, axis=0)  # (D, B, S) bf16
    return np.ascontiguousarray(y_full.transpose(1, 2, 0)).astype(np.float32)


def kernel(x, w):
    from concourse.bass_utils import run_bass_kernel_spmd

    nc = _get_nc()
    in_maps = _make_in_maps(x, w)
    trace = bool(int(os.environ.get("KERNEL_TRACE", "0")))
    res = None
    err = None
    for attempt in range(3):
        try:
            res = run_bass_kernel_spmd(
                nc, in_maps, core_ids=list(range(N_CORES)),
                trace=trace and attempt == 0,
            )
            break
        except Exception as e:  # transient NRT device errors / missing trace hook
            err = e
            os.environ["BASS_NEVER_TRACE"] = "1"
            trace = False
    if res is None:
        raise err
    _CACHE["last_results"] = res
    return _assemble(res.results)
